# revision 40
# baseline (speedup 1.0000x reference)
"""End2EndPoseLoss on 8 Trainium2 NeuronCores.

Data-parallel over batch (B_LOC=2 samples/core). Key structural wins:

1. Mask sparsity, exact: a row's whole heatmap contribution is
   mask * stuff and mask is a kernel input, so each core gathers ONLY
   its mask=1 rows (padded to the max across cores, rounded to 64;
   module compiled per row-count). Seed-0 inputs: 512 of 680 rows.
2. Mask applied on device for free: each g piece carries 2 extra fp16
   columns (mask, 2*mask) used as the ACT Square scale / DVE stt
   scalar, so padding rows vanish and no per-row output is needed.
3. Tiny output: masked sums cross-partition-reduced on the idle PE
   (ones-vector matmul -> PSUM [1, nsum]) and shipped together with
   the small-loss terms as ONE [2, nsum+3] tensor (2 descriptors).
   Avoids multi-descriptor outputs whose completion-semaphore drain
   costs ~25ns/tick at kernel end.
4. Column-halved [128, 2048] pieces: first ACT work starts ~8us
   earlier and the final dependency chain is short.

Per piece: DVE step=(g>0.2) (TS 4x), d=p-g (TT 2x), n=step*d (TT 2x);
ACT Square(mask*d)+accum -> col 2i; col 2i+1 = ACT Square(2mask*n) or,
for the last two pieces, DVE stt (n*2mask)*n+accum (host doubles those
columns) — balances DVE vs ACT busy time.

Measured constraints that shaped this (HW, not the cost model):
  - ts/stt-with-accum run at ~1x (TENSOR_SCALAR_CACHE_REDUCE), never 4x
  - the Pool engine only supports tensor_tensor Add/Mult (no TS/STT),
    and big Pool ops starve DVE via SBUF contention (~2x slowdowns)
  - ACT is exactly 1 elem/clk/lane @1.2GHz regardless of dtype
  - ~7.4us of end-of-kernel semaphore teardown is counted in exec time
    (constant, unavoidable)

Small losses (count CE over [2,21], conf focal over [2,20]) arrive as
one packed [2,84] tensor and run inside the DMA stream.
"""

import sys
import types
import numpy as np

import concourse.bacc as bacc
import concourse.bass as bass  # noqa: F401
import concourse.mybir as mybir
import concourse.tile as tile
from concourse import bass_utils

# Problem constants (hardcoded per contract).
B, P, K, H, W = 16, 20, 17, 64, 64
N_CORES = 8
B_LOC = B // N_CORES            # 2
ROWS = B_LOC * P * K            # 680
COLS = H * W                    # 4096
REM = 40                        # 680 - 5*128

PEAK_THRESH = 0.2
PEAK_WEIGHT = 5.0
ALPHA_COUNT, ALPHA_HEATMAP, ALPHA_CONF = 1.0, 10.0, 1.5
EPS = 1e-6

F32 = mybir.dt.float32
F16 = mybir.dt.float16
ALU = mybir.AluOpType
ACTF = mybir.ActivationFunctionType
AX = mybir.AxisListType

# Only rows with mask=1 are shipped (a row's entire contribution is
# exactly mask * stuff, and mask is a kernel input): each core gathers
# its active rows, padded to the max across cores rounded up to 64.
# pieces: (name, rows, cols, s2_dve = Sum2 via DVE stt instead of ACT
# square); piece i accumulates Sum(d^2) in column 2i, scaled
# Sum(step*d^2) in 2i+1. NROWS % 128 == 64 folds the last 64 rows to
# [128, 2048] (partition h*64+r <-> row r, column half h).
PIECES = []
NSUM = 0


def pieces_for(nrows):
    """Piece list: (name, rows, cols, s2_dve, row0, col0, folded).
    Column-halved [128, 2048] pieces: first data lands sooner and the
    final dependency chain is short. Sum2 runs on DVE stt for the last
    two pieces (~1 chunk-unit), ACT Square otherwise."""
    nf, rem = divmod(nrows, 128)
    assert rem in (0, 64)
    ps = []
    for k in range(nf):
        for h in range(2):
            ps.append(("c%d_%d" % (k, h), 128, COLS // 2, False,
                       k * 128, h * (COLS // 2), False))
    if rem:
        ps.append(("r", 128, COLS // 2, False, nf * 128, 0, True))
    for j in range(max(0, len(ps) - 2), len(ps)):
        nm, rr, cc, _, r0, c0, fo = ps[j]
        ps[j] = (nm, rr, cc, True, r0, c0, fo)
    return ps


def _install_ntff_hook():
    """Provide antenv.axon_hooks if the image lacks it, so that
    run_bass_kernel_spmd(trace=True) doesn't crash and, when possible,
    actually profiles via the axon .so."""
    try:
        from antenv.axon_hooks import get_axon_ntff_profile_hook  # noqa: F401
        return
    except ImportError:
        pass
    try:
        import antenv
    except ImportError:
        return
    import contextlib
    import ctypes

    mod = types.ModuleType("antenv.axon_hooks")
    _h = [None]
    mod.set_axon_ntff_profile_hook = lambda h: _h.__setitem__(0, h)
    mod.get_axon_ntff_profile_hook = lambda: _h[0]
    sys.modules["antenv.axon_hooks"] = mod
    antenv.axon_hooks = mod

    so_path = "/opt/axon/libaxon_pjrt.so"
    try:
        lib = ctypes.CDLL(so_path)
        if not hasattr(lib, "axon_start_nrt_profile"):
            return
        lib.axon_start_nrt_profile.argtypes = [
            ctypes.POINTER(ctypes.c_int64),
            ctypes.c_size_t,
        ]
        lib.axon_start_nrt_profile.restype = ctypes.c_int64
        lib.axon_stop_nrt_profile.argtypes = [ctypes.c_char_p]
        lib.axon_stop_nrt_profile.restype = ctypes.c_int64
    except OSError:
        return

    @contextlib.contextmanager
    def _hook(output_dir, device_ids):
        import jax

        jax.devices()
        if device_ids:
            ids = (ctypes.c_int64 * len(device_ids))(*device_ids)
            rc = lib.axon_start_nrt_profile(ids, len(device_ids))
        else:
            rc = lib.axon_start_nrt_profile(None, 0)
        if rc != 0:
            raise RuntimeError(f"axon_start_nrt_profile rc={rc}")
        try:
            yield
        finally:
            n = lib.axon_stop_nrt_profile(str(output_dir).encode())
            print(f"profile: {n} file(s) written to {output_dir}", file=sys.stderr)

    mod.set_axon_ntff_profile_hook(_hook)


_install_ntff_hook()

_orig_upload = bass_utils.upload_artifacts


def _safe_upload(tmpdir):
    try:
        return _orig_upload(tmpdir)
    except Exception:
        return tmpdir


bass_utils.upload_artifacts = _safe_upload


def build_module(pieces):
    nsum = max(2 * len(pieces), 1)
    nc = bacc.Bacc("TRN2", target_bir_lowering=False, debug=False)

    srcs = {}
    for name, rr, cc, _, _, _, _ in pieces:
        # g pieces carry 2 extra columns: mask and 2*mask for that
        # partition's row (applied on device as ACT scale / stt scalar)
        srcs["g" + name] = nc.dram_tensor("g" + name, [rr, cc + 2], F16, kind="ExternalInput")
        srcs["p" + name] = nc.dram_tensor("p" + name, [rr, cc], F16, kind="ExternalInput")
    sm = nc.dram_tensor("sm", [B_LOC, 84], F32, kind="ExternalInput")

    # single tiny output: row 0 cols 0:nsum = mask-weighted piece sums,
    # cols nsum:nsum+3 (both rows) = pre, ln(se), focal accum
    out_fin = nc.dram_tensor("out_fin", [B_LOC, nsum + 3], F32, kind="ExternalOutput")

    with tile.TileContext(nc) as tc:
        with (
            tc.tile_pool(name="io", bufs=5) as io,
            tc.tile_pool(name="wk", bufs=4) as wk,
            tc.tile_pool(name="acc", bufs=1) as accp,
            tc.tile_pool(name="small", bufs=1) as small,
            tc.tile_pool(name="ps", bufs=1, space=bass.MemorySpace.PSUM) as ps,
        ):
            sums = accp.tile([128, nsum], F32, tag="sums")
            ones = accp.tile([128, 1], F32, tag="ones")
            psum_s = ps.tile([1, nsum], F32, tag="pt")
            nc.gpsimd.memset(sums[:], 0.0)
            nc.gpsimd.memset(ones[:], 1.0)

            # ---- input DMA triggers, g before p so W4 can start early ----
            gts, pts = [], []
            for i, (name, rr, cc, _, _, _, _) in enumerate(pieces):
                g = io.tile([128, COLS + 2], F16, tag="g")
                p = io.tile([128, COLS], F16, tag="p")
                nc.sync.dma_start(g[:rr, :cc + 2], srcs["g" + name][:, :])
                nc.sync.dma_start(p[:rr, :cc], srcs["p" + name][:, :])
                gts.append(g)
                pts.append(p)
                if i == 0:
                    smt = small.tile([B_LOC, 84], F32, tag="sm")
                    nc.sync.dma_start(smt[:], sm[:, :])

            def heavy(i):
                name, rr, cc, s2_dve = pieces[i][:4]
                c1, c2 = 2 * i, 2 * i + 1
                g, p = gts[i], pts[i]
                st = wk.tile([128, COLS], F16, tag="st")
                d = wk.tile([128, COLS], F16, tag="d")
                n = wk.tile([128, COLS], F16, tag="n")
                # step = (g > thresh)          (TS, 4x)
                nc.vector.tensor_scalar(
                    st[:rr, :cc], g[:rr, :cc], float(PEAK_THRESH), None,
                    op0=ALU.is_gt,
                )
                # d = p - g                    (TT, 2x)
                nc.vector.tensor_sub(d[:rr, :cc], p[:rr, :cc], g[:rr, :cc])
                # n = step * d                 (TT, 2x)
                nc.vector.tensor_mul(n[:rr, :cc], st[:rr, :cc], d[:rr, :cc])
                mk = wk.tile([128, 2], F32, tag="mk")
                nc.vector.tensor_copy(mk[:rr, :], g[:rr, cc : cc + 2])
                mcol = mk[:rr, 0:1]               # mask {0,1}
                m2col = mk[:rr, 1:2]              # 2*mask {0,2}
                # mask*Sum(d^2) on ACT: Square(mask*d), accumulator -> col 2i
                nc.scalar.activation(
                    d[:rr, :cc], d[:rr, :cc], ACTF.Square, scale=mcol,
                    accum_out=sums[:rr, c1 : c1 + 1],
                )
                # col 2i+1: ACT Square(2*mask*n) = 4*mask*Sum(n^2), or DVE
                # stt (n*2mask)*n = 2*mask*Sum(n^2) (host doubles those cols)
                if s2_dve:
                    nc.vector.scalar_tensor_tensor(
                        out=g[:rr, :cc], in0=n[:rr, :cc], scalar=m2col,
                        in1=n[:rr, :cc], op0=ALU.mult, op1=ALU.mult,
                        accum_out=sums[:rr, c2 : c2 + 1],
                    )
                else:
                    nc.scalar.activation(
                        n[:rr, :cc], n[:rr, :cc], ACTF.Square, scale=m2col,
                        accum_out=sums[:rr, c2 : c2 + 1],
                    )

            # first pieces keep DVE/ACT fed through the ramp; the
            # small losses interleave after them
            nearly = min(3, len(pieces))
            for i in range(nearly):
                heavy(i)

            # ---- small losses (inside the DMA stream) ----
            cl_t = smt[:, 0:21]
            oh_t = smt[:, 21:42]
            lt_ = smt[:, 42:62]
            tt_ = smt[:, 62:82]

            # count cross-entropy pieces
            mx = small.tile([B_LOC, 1], F32, tag="mx")
            nc.vector.tensor_reduce(mx[:], cl_t, axis=AX.X, op=ALU.max)
            nmx = small.tile([B_LOC, 1], F32, tag="nmx")
            nc.vector.tensor_scalar_mul(nmx[:], mx[:], -1.0)
            et = small.tile([B_LOC, 21], F32, tag="et")
            se = small.tile([B_LOC, 1], F32, tag="se")
            nc.scalar.activation(
                et[:], cl_t, ACTF.Exp, bias=nmx[:], scale=1.0, accum_out=se[:]
            )
            junk21 = small.tile([B_LOC, 21], F32, tag="junk21")
            tg = small.tile([B_LOC, 1], F32, tag="tg")
            nc.vector.scalar_tensor_tensor(
                out=junk21[:], in0=cl_t, scalar=1.0, in1=oh_t,
                op0=ALU.mult, op1=ALU.mult, accum_out=tg[:],
            )
            fin = small.tile([B_LOC, nsum + 3], F32, tag="fin")
            nc.gpsimd.memset(fin[:], 0.0)
            nc.vector.tensor_sub(fin[:, nsum : nsum + 1], mx[:], tg[:])

            # focal: p_t = 1 - |t - sigma(l)| with sigma from exp(-|l|)
            ab = small.tile([B_LOC, P], F32, tag="ab")
            nc.vector.scalar_tensor_tensor(
                out=ab[:], in0=lt_, scalar=-1.0, in1=lt_,
                op0=ALU.mult, op1=ALU.max,
            )
            z = small.tile([B_LOC, P], F32, tag="z")
            nc.scalar.activation(z[:], ab[:], ACTF.Exp, scale=-1.0)
            zz = small.tile([B_LOC, P], F32, tag="zz")
            nc.vector.tensor_scalar(zz[:], z[:], 1.0, None, op0=ALU.add)
            r = small.tile([B_LOC, P], F32, tag="r")
            nc.vector.reciprocal(r[:], zz[:])          # sigma(|l|)
            sgn = small.tile([B_LOC, P], F32, tag="sgn")
            nc.vector.tensor_scalar(sgn[:], lt_, 0.0, None, op0=ALU.is_ge)
            t1 = small.tile([B_LOC, P], F32, tag="t1")
            nc.vector.tensor_scalar(t1[:], r[:], 2.0, -1.0, op0=ALU.mult, op1=ALU.add)
            t2 = small.tile([B_LOC, P], F32, tag="t2")
            nc.vector.tensor_scalar(t2[:], r[:], -1.0, 1.0, op0=ALU.mult, op1=ALU.add)
            sl0 = small.tile([B_LOC, P], F32, tag="sl0")
            nc.vector.scalar_tensor_tensor(
                out=sl0[:], in0=sgn[:], scalar=1.0, in1=t1[:],
                op0=ALU.mult, op1=ALU.mult,
            )
            sig = small.tile([B_LOC, P], F32, tag="sig")
            nc.gpsimd.tensor_add(sig[:], sl0[:], t2[:])
            u = small.tile([B_LOC, P], F32, tag="u")
            nc.gpsimd.tensor_sub(u[:], tt_, sig[:])
            au = small.tile([B_LOC, P], F32, tag="au")
            nc.vector.scalar_tensor_tensor(
                out=au[:], in0=u[:], scalar=-1.0, in1=u[:],
                op0=ALU.mult, op1=ALU.max,
            )
            pt = small.tile([B_LOC, P], F32, tag="pt")
            nc.vector.tensor_scalar(pt[:], au[:], -1.0, 1.0, op0=ALU.mult, op1=ALU.add)
            au2 = small.tile([B_LOC, P], F32, tag="au2")
            nc.gpsimd.tensor_mul(au2[:], au[:], au[:])

            lnz = small.tile([B_LOC, 1], F32, tag="lnz")
            nc.scalar.activation(lnz[:], se[:], ACTF.Ln)
            nc.vector.tensor_copy(fin[:, nsum + 1 : nsum + 2], lnz[:])
            lnpt = small.tile([B_LOC, P], F32, tag="lnpt")
            nc.scalar.activation(lnpt[:], pt[:], ACTF.Ln)
            junk20 = small.tile([B_LOC, P], F32, tag="junk20")
            fr = small.tile([B_LOC, 1], F32, tag="fr")
            # accum = sum(au^2 * ln(p_t)) = -focal_sum   (host negates)
            nc.vector.scalar_tensor_tensor(
                out=junk20[:], in0=au2[:], scalar=1.0, in1=lnpt[:],
                op0=ALU.mult, op1=ALU.mult, accum_out=fr[:],
            )
            nc.vector.tensor_copy(fin[:, nsum + 2 : nsum + 3], fr[:])

            # ---- remaining heavy pieces ----
            for i in range(nearly, len(pieces)):
                heavy(i)

            # ---- cross-partition reduce of masked sums on PE, 1 tiny DMA ----
            if pieces:
                nc.tensor.matmul(psum_s[0:1, :], ones[:, :], sums[:, :])
                nc.vector.tensor_copy(fin[0:1, 0:nsum], psum_s[0:1, :])
            nc.sync.dma_start(out_fin[:, :], fin[:, :])

    nc.compile()
    return nc


_MODULES = {}


def _module(nrows):
    global PIECES, NSUM
    PIECES = pieces_for(nrows)
    NSUM = max(2 * len(PIECES), 1)
    if nrows not in _MODULES:
        _MODULES[nrows] = build_module(PIECES)
    return _MODULES[nrows]


def nrows_for_mask(mask):
    mask = np.asarray(mask)
    mx = max(int(mask[i * B_LOC : (i + 1) * B_LOC].sum()) * K
             for i in range(N_CORES))
    return ((mx + 63) // 64) * 64


def make_in_maps(count_logits, pred_heatmaps, pred_conf_logits, gt_heatmaps,
                 count, mask, pieces, nrows):
    count_logits = np.asarray(count_logits, np.float32)
    pred_heatmaps = np.asarray(pred_heatmaps, np.float32)
    pred_conf_logits = np.asarray(pred_conf_logits, np.float32)
    gt_heatmaps = np.asarray(gt_heatmaps, np.float32)
    count = np.asarray(count, np.int32)
    mask = np.asarray(mask, np.int32)

    in_maps = []
    for i in range(N_CORES):
        b0, b1 = i * B_LOC, (i + 1) * B_LOC
        mrow = np.repeat(mask[b0:b1].reshape(-1), K)          # [680]
        act = np.nonzero(mrow)[0]
        # gather active rows, pad with zero rows (mask col 0) to nrows
        phl = np.zeros((nrows, COLS), np.float16)
        ghl = np.zeros((nrows, COLS), np.float16)
        mcol = np.zeros((nrows,), np.float16)
        na = len(act)
        phl[:na] = pred_heatmaps[b0:b1].reshape(ROWS, COLS)[act]
        ghl[:na] = gt_heatmaps[b0:b1].reshape(ROWS, COLS)[act]
        mcol[:na] = 1.0

        im = {}
        for name, rr, cc, _, r0, c0, folded in pieces:
            if folded:
                # last 64 rows folded [128, 2048]: partition h*64+r
                pr = phl[r0 : r0 + 64].reshape(64, 2, cc).transpose(1, 0, 2)
                gr = ghl[r0 : r0 + 64].reshape(64, 2, cc).transpose(1, 0, 2)
                pch = pr.reshape(128, cc)
                gch = gr.reshape(128, cc)
                mv = np.concatenate([mcol[r0 : r0 + 64]] * 2)[:, None]
            else:
                pch = phl[r0 : r0 + rr, c0 : c0 + cc]
                gch = ghl[r0 : r0 + rr, c0 : c0 + cc]
                mv = mcol[r0 : r0 + rr][:, None]
            im["p" + name] = np.ascontiguousarray(pch)
            im["g" + name] = np.ascontiguousarray(np.concatenate(
                [gch, mv, 2.0 * mv], axis=1).astype(np.float16))

        smv = np.zeros((B_LOC, 84), np.float32)
        smv[np.arange(B_LOC), 21 + count[b0:b1]] = 1.0       # one-hot
        smv[:, 0:21] = count_logits[b0:b1]
        smv[:, 42:62] = pred_conf_logits[b0:b1]
        smv[:, 62:82] = mask[b0:b1].astype(np.float32)
        im["sm"] = smv
        in_maps.append(im)
    return in_maps


def combine(results, mask):
    mask = np.asarray(mask)
    # stt pieces accumulated 2*mask*Sum(n^2); double those columns
    kappa = np.ones(NSUM)
    for i, (_, _, _, s2_dve, _, _, _) in enumerate(PIECES):
        if s2_dve:
            kappa[2 * i + 1] = 2.0
    hm_sum = 0.0
    ce_sum = 0.0
    fo_sum = 0.0
    ns = 2 * len(PIECES)
    for res in results:
        fin = np.asarray(res["out_fin"], np.float64)
        if ns:
            hm_sum += float(fin[0, :ns] @ kappa[:ns])
        ce_sum += float(fin[:, -3].sum() + fin[:, -2].sum())
        fo_sum += -float(fin[:, -1].sum())
    msum = float(mask.sum())
    hm = hm_sum / (msum * K * H * W + EPS)
    loss_heatmap = hm if msum > 0 else 0.0
    loss_count = ce_sum / B
    loss_conf = fo_sum / (B * P)
    total = (ALPHA_COUNT * loss_count + ALPHA_HEATMAP * loss_heatmap
             + ALPHA_CONF * loss_conf)
    return np.float32(total)


def run(inputs, trace=False, **kwargs):
    """Run on hardware; returns (output_scalar, BassKernelResults)."""
    nrows = nrows_for_mask(inputs["mask"])
    nc = _module(nrows)
    in_maps = make_in_maps(pieces=PIECES, nrows=nrows, **inputs)
    res = bass_utils.run_bass_kernel_spmd(
        nc, in_maps, core_ids=list(range(N_CORES)), trace=trace, **kwargs
    )
    out = combine(res.results, inputs["mask"])
    return out, res


def kernel(count_logits, pred_heatmaps, pred_conf_logits, gt_heatmaps,
           count, mask):
    out, _ = run(dict(
        count_logits=count_logits, pred_heatmaps=pred_heatmaps,
        pred_conf_logits=pred_conf_logits, gt_heatmaps=gt_heatmaps,
        count=count, mask=mask,
    ))
    return out


# revision 41
# speedup vs baseline: 1.0008x; 1.0008x over previous
"""End2EndPoseLoss on 8 Trainium2 NeuronCores.

Data-parallel over batch (B_LOC=2 samples/core). Key structural wins:

1. Mask sparsity, exact: a row's whole heatmap contribution is
   mask * stuff and mask is a kernel input, so each core gathers ONLY
   its mask=1 rows (padded to the max across cores, rounded to 64;
   module compiled per row-count). Seed-0 inputs: 512 of 680 rows.
2. Mask applied on device for free: each g piece carries 2 extra fp16
   columns (mask, 2*mask) used as the ACT Square scale / DVE stt
   scalar, so padding rows vanish and no per-row output is needed.
3. Tiny output: masked sums cross-partition-reduced on the idle PE
   (ones-vector matmul -> PSUM [1, nsum]) and shipped together with
   the small-loss terms as ONE [2, nsum+3] tensor (2 descriptors).
   Avoids multi-descriptor outputs whose completion-semaphore drain
   costs ~25ns/tick at kernel end.
4. Column-halved [128, 2048] pieces: first ACT work starts ~8us
   earlier and the final dependency chain is short.

Per piece: DVE step=(g>0.2) (TS 4x), d=p-g (TT 2x), n=step*d (TT 2x);
ACT Square(mask*d)+accum -> col 2i; col 2i+1 = ACT Square(2mask*n) or,
for the last two pieces, DVE stt (n*2mask)*n+accum (host doubles those
columns) — balances DVE vs ACT busy time.

Measured constraints that shaped this (HW, not the cost model):
  - ts/stt-with-accum run at ~1x (TENSOR_SCALAR_CACHE_REDUCE), never 4x
  - the Pool engine only supports tensor_tensor Add/Mult (no TS/STT),
    and big Pool ops starve DVE via SBUF contention (~2x slowdowns)
  - ACT is exactly 1 elem/clk/lane @1.2GHz regardless of dtype
  - ~7.4us of end-of-kernel semaphore teardown is counted in exec time
    (constant, unavoidable)

Small losses (count CE over [2,21], conf focal over [2,20]) arrive as
one packed [2,84] tensor and run inside the DMA stream.
"""

import sys
import types
import numpy as np

import concourse.bacc as bacc
import concourse.bass as bass  # noqa: F401
import concourse.mybir as mybir
import concourse.tile as tile
from concourse import bass_utils

# Problem constants (hardcoded per contract).
B, P, K, H, W = 16, 20, 17, 64, 64
N_CORES = 8
B_LOC = B // N_CORES            # 2
ROWS = B_LOC * P * K            # 680
COLS = H * W                    # 4096
REM = 40                        # 680 - 5*128

PEAK_THRESH = 0.2
PEAK_WEIGHT = 5.0
ALPHA_COUNT, ALPHA_HEATMAP, ALPHA_CONF = 1.0, 10.0, 1.5
EPS = 1e-6

F32 = mybir.dt.float32
F16 = mybir.dt.float16
ALU = mybir.AluOpType
ACTF = mybir.ActivationFunctionType
AX = mybir.AxisListType

# Only rows with mask=1 are shipped (a row's entire contribution is
# exactly mask * stuff, and mask is a kernel input): each core gathers
# its active rows, padded to the max across cores rounded up to 64.
# pieces: (name, rows, cols, s2_dve = Sum2 via DVE stt instead of ACT
# square); piece i accumulates Sum(d^2) in column 2i, scaled
# Sum(step*d^2) in 2i+1. NROWS % 128 == 64 folds the last 64 rows to
# [128, 2048] (partition h*64+r <-> row r, column half h).
PIECES = []
NSUM = 0


def pieces_for(nrows):
    """Piece list: (name, rows, cols, s2_dve, row0, col0, folded).
    Column-halved [128, 2048] pieces: first data lands sooner and the
    final dependency chain is short. Sum2 runs on DVE stt for the last
    two pieces (~1 chunk-unit), ACT Square otherwise."""
    nf, rem = divmod(nrows, 128)
    assert rem in (0, 64)
    ps = []
    for k in range(nf):
        for h in range(2):
            ps.append(("c%d_%d" % (k, h), 128, COLS // 2, False,
                       k * 128, h * (COLS // 2), False))
    if rem:
        ps.append(("r", 128, COLS // 2, False, nf * 128, 0, True))
    for j in range(max(0, len(ps) - 2), len(ps)):
        nm, rr, cc, _, r0, c0, fo = ps[j]
        ps[j] = (nm, rr, cc, True, r0, c0, fo)
    return ps


def _install_ntff_hook():
    """Provide antenv.axon_hooks if the image lacks it, so that
    run_bass_kernel_spmd(trace=True) doesn't crash and, when possible,
    actually profiles via the axon .so."""
    try:
        from antenv.axon_hooks import get_axon_ntff_profile_hook  # noqa: F401
        return
    except ImportError:
        pass
    try:
        import antenv
    except ImportError:
        return
    import contextlib
    import ctypes

    mod = types.ModuleType("antenv.axon_hooks")
    _h = [None]
    mod.set_axon_ntff_profile_hook = lambda h: _h.__setitem__(0, h)
    mod.get_axon_ntff_profile_hook = lambda: _h[0]
    sys.modules["antenv.axon_hooks"] = mod
    antenv.axon_hooks = mod

    so_path = "/opt/axon/libaxon_pjrt.so"
    try:
        lib = ctypes.CDLL(so_path)
        if not hasattr(lib, "axon_start_nrt_profile"):
            return
        lib.axon_start_nrt_profile.argtypes = [
            ctypes.POINTER(ctypes.c_int64),
            ctypes.c_size_t,
        ]
        lib.axon_start_nrt_profile.restype = ctypes.c_int64
        lib.axon_stop_nrt_profile.argtypes = [ctypes.c_char_p]
        lib.axon_stop_nrt_profile.restype = ctypes.c_int64
    except OSError:
        return

    @contextlib.contextmanager
    def _hook(output_dir, device_ids):
        import jax

        jax.devices()
        if device_ids:
            ids = (ctypes.c_int64 * len(device_ids))(*device_ids)
            rc = lib.axon_start_nrt_profile(ids, len(device_ids))
        else:
            rc = lib.axon_start_nrt_profile(None, 0)
        if rc != 0:
            raise RuntimeError(f"axon_start_nrt_profile rc={rc}")
        try:
            yield
        finally:
            n = lib.axon_stop_nrt_profile(str(output_dir).encode())
            print(f"profile: {n} file(s) written to {output_dir}", file=sys.stderr)

    mod.set_axon_ntff_profile_hook(_hook)


_install_ntff_hook()

_orig_upload = bass_utils.upload_artifacts


def _safe_upload(tmpdir):
    try:
        return _orig_upload(tmpdir)
    except Exception:
        return tmpdir


bass_utils.upload_artifacts = _safe_upload


def build_module(pieces):
    nsum = max(2 * len(pieces), 1)
    nc = bacc.Bacc("TRN2", target_bir_lowering=False, debug=False)

    srcs = {}
    for name, rr, cc, _, _, _, _ in pieces:
        # g pieces carry 2 extra columns: mask and 2*mask for that
        # partition's row (applied on device as ACT scale / stt scalar)
        srcs["g" + name] = nc.dram_tensor("g" + name, [rr, cc + 2], F16, kind="ExternalInput")
        srcs["p" + name] = nc.dram_tensor("p" + name, [rr, cc], F16, kind="ExternalInput")
    sm = nc.dram_tensor("sm", [B_LOC, 84], F32, kind="ExternalInput")

    # single tiny output: row 0 cols 0:nsum = mask-weighted piece sums,
    # cols nsum:nsum+3 (both rows) = pre, ln(se), focal accum
    out_fin = nc.dram_tensor("out_fin", [B_LOC, nsum + 3], F32, kind="ExternalOutput")

    with tile.TileContext(nc) as tc:
        with (
            tc.tile_pool(name="io", bufs=5) as io,
            tc.tile_pool(name="wk", bufs=4) as wk,
            tc.tile_pool(name="acc", bufs=1) as accp,
            tc.tile_pool(name="small", bufs=1) as small,
            tc.tile_pool(name="ps", bufs=1, space=bass.MemorySpace.PSUM) as ps,
        ):
            sums = accp.tile([128, nsum], F32, tag="sums")
            ones = accp.tile([128, 1], F32, tag="ones")
            psum_s = ps.tile([1, nsum], F32, tag="pt")
            nc.gpsimd.memset(sums[:], 0.0)
            nc.gpsimd.memset(ones[:], 1.0)
            # Dummy Exp/Ln on one element: forces both ACT tables (each
            # set also contains Square) to load during the DMA ramp
            # instead of stalling the ACT stream later.
            scr = accp.tile([1, 2], F32, tag="scr")
            nc.scalar.activation(scr[0:1, 0:1], ones[0:1, 0:1], ACTF.Exp)
            nc.scalar.activation(scr[0:1, 1:2], ones[0:1, 0:1], ACTF.Ln)

            # ---- input DMA triggers, g before p so W4 can start early ----
            gts, pts = [], []
            for i, (name, rr, cc, _, _, _, _) in enumerate(pieces):
                g = io.tile([128, COLS + 2], F16, tag="g")
                p = io.tile([128, COLS], F16, tag="p")
                nc.sync.dma_start(g[:rr, :cc + 2], srcs["g" + name][:, :])
                nc.sync.dma_start(p[:rr, :cc], srcs["p" + name][:, :])
                gts.append(g)
                pts.append(p)
                if i == 0:
                    smt = small.tile([B_LOC, 84], F32, tag="sm")
                    nc.sync.dma_start(smt[:], sm[:, :])

            def heavy(i):
                name, rr, cc, s2_dve = pieces[i][:4]
                c1, c2 = 2 * i, 2 * i + 1
                g, p = gts[i], pts[i]
                st = wk.tile([128, COLS], F16, tag="st")
                d = wk.tile([128, COLS], F16, tag="d")
                n = wk.tile([128, COLS], F16, tag="n")
                # step = (g > thresh)          (TS, 4x)
                nc.vector.tensor_scalar(
                    st[:rr, :cc], g[:rr, :cc], float(PEAK_THRESH), None,
                    op0=ALU.is_gt,
                )
                # d = p - g                    (TT, 2x)
                nc.vector.tensor_sub(d[:rr, :cc], p[:rr, :cc], g[:rr, :cc])
                # n = step * d                 (TT, 2x)
                nc.vector.tensor_mul(n[:rr, :cc], st[:rr, :cc], d[:rr, :cc])
                mk = wk.tile([128, 2], F32, tag="mk")
                nc.vector.tensor_copy(mk[:rr, :], g[:rr, cc : cc + 2])
                mcol = mk[:rr, 0:1]               # mask {0,1}
                m2col = mk[:rr, 1:2]              # 2*mask {0,2}
                # mask*Sum(d^2) on ACT: Square(mask*d), accumulator -> col 2i
                nc.scalar.activation(
                    d[:rr, :cc], d[:rr, :cc], ACTF.Square, scale=mcol,
                    accum_out=sums[:rr, c1 : c1 + 1],
                )
                # col 2i+1: ACT Square(2*mask*n) = 4*mask*Sum(n^2), or DVE
                # stt (n*2mask)*n = 2*mask*Sum(n^2) (host doubles those cols)
                if s2_dve:
                    nc.vector.scalar_tensor_tensor(
                        out=g[:rr, :cc], in0=n[:rr, :cc], scalar=m2col,
                        in1=n[:rr, :cc], op0=ALU.mult, op1=ALU.mult,
                        accum_out=sums[:rr, c2 : c2 + 1],
                    )
                else:
                    nc.scalar.activation(
                        n[:rr, :cc], n[:rr, :cc], ACTF.Square, scale=m2col,
                        accum_out=sums[:rr, c2 : c2 + 1],
                    )

            # first pieces keep DVE/ACT fed through the ramp; the
            # small losses interleave after them
            nearly = min(3, len(pieces))
            for i in range(nearly):
                heavy(i)

            # ---- small losses (inside the DMA stream) ----
            cl_t = smt[:, 0:21]
            oh_t = smt[:, 21:42]
            lt_ = smt[:, 42:62]
            tt_ = smt[:, 62:82]

            # count cross-entropy pieces
            mx = small.tile([B_LOC, 1], F32, tag="mx")
            nc.vector.tensor_reduce(mx[:], cl_t, axis=AX.X, op=ALU.max)
            nmx = small.tile([B_LOC, 1], F32, tag="nmx")
            nc.vector.tensor_scalar_mul(nmx[:], mx[:], -1.0)
            et = small.tile([B_LOC, 21], F32, tag="et")
            se = small.tile([B_LOC, 1], F32, tag="se")
            nc.scalar.activation(
                et[:], cl_t, ACTF.Exp, bias=nmx[:], scale=1.0, accum_out=se[:]
            )
            junk21 = small.tile([B_LOC, 21], F32, tag="junk21")
            tg = small.tile([B_LOC, 1], F32, tag="tg")
            nc.vector.scalar_tensor_tensor(
                out=junk21[:], in0=cl_t, scalar=1.0, in1=oh_t,
                op0=ALU.mult, op1=ALU.mult, accum_out=tg[:],
            )
            fin = small.tile([B_LOC, nsum + 3], F32, tag="fin")
            nc.gpsimd.memset(fin[:], 0.0)
            nc.vector.tensor_sub(fin[:, nsum : nsum + 1], mx[:], tg[:])

            # focal: p_t = 1 - |t - sigma(l)| with sigma from exp(-|l|)
            ab = small.tile([B_LOC, P], F32, tag="ab")
            nc.vector.scalar_tensor_tensor(
                out=ab[:], in0=lt_, scalar=-1.0, in1=lt_,
                op0=ALU.mult, op1=ALU.max,
            )
            z = small.tile([B_LOC, P], F32, tag="z")
            nc.scalar.activation(z[:], ab[:], ACTF.Exp, scale=-1.0)
            zz = small.tile([B_LOC, P], F32, tag="zz")
            nc.vector.tensor_scalar(zz[:], z[:], 1.0, None, op0=ALU.add)
            r = small.tile([B_LOC, P], F32, tag="r")
            nc.vector.reciprocal(r[:], zz[:])          # sigma(|l|)
            sgn = small.tile([B_LOC, P], F32, tag="sgn")
            nc.vector.tensor_scalar(sgn[:], lt_, 0.0, None, op0=ALU.is_ge)
            t1 = small.tile([B_LOC, P], F32, tag="t1")
            nc.vector.tensor_scalar(t1[:], r[:], 2.0, -1.0, op0=ALU.mult, op1=ALU.add)
            t2 = small.tile([B_LOC, P], F32, tag="t2")
            nc.vector.tensor_scalar(t2[:], r[:], -1.0, 1.0, op0=ALU.mult, op1=ALU.add)
            sl0 = small.tile([B_LOC, P], F32, tag="sl0")
            nc.vector.scalar_tensor_tensor(
                out=sl0[:], in0=sgn[:], scalar=1.0, in1=t1[:],
                op0=ALU.mult, op1=ALU.mult,
            )
            sig = small.tile([B_LOC, P], F32, tag="sig")
            nc.gpsimd.tensor_add(sig[:], sl0[:], t2[:])
            u = small.tile([B_LOC, P], F32, tag="u")
            nc.gpsimd.tensor_sub(u[:], tt_, sig[:])
            au = small.tile([B_LOC, P], F32, tag="au")
            nc.vector.scalar_tensor_tensor(
                out=au[:], in0=u[:], scalar=-1.0, in1=u[:],
                op0=ALU.mult, op1=ALU.max,
            )
            pt = small.tile([B_LOC, P], F32, tag="pt")
            nc.vector.tensor_scalar(pt[:], au[:], -1.0, 1.0, op0=ALU.mult, op1=ALU.add)
            au2 = small.tile([B_LOC, P], F32, tag="au2")
            nc.gpsimd.tensor_mul(au2[:], au[:], au[:])

            lnz = small.tile([B_LOC, 1], F32, tag="lnz")
            nc.scalar.activation(lnz[:], se[:], ACTF.Ln)
            nc.vector.tensor_copy(fin[:, nsum + 1 : nsum + 2], lnz[:])
            lnpt = small.tile([B_LOC, P], F32, tag="lnpt")
            nc.scalar.activation(lnpt[:], pt[:], ACTF.Ln)
            junk20 = small.tile([B_LOC, P], F32, tag="junk20")
            fr = small.tile([B_LOC, 1], F32, tag="fr")
            # accum = sum(au^2 * ln(p_t)) = -focal_sum   (host negates)
            nc.vector.scalar_tensor_tensor(
                out=junk20[:], in0=au2[:], scalar=1.0, in1=lnpt[:],
                op0=ALU.mult, op1=ALU.mult, accum_out=fr[:],
            )
            nc.vector.tensor_copy(fin[:, nsum + 2 : nsum + 3], fr[:])

            # ---- remaining heavy pieces ----
            for i in range(nearly, len(pieces)):
                heavy(i)

            # ---- cross-partition reduce of masked sums on PE, 1 tiny DMA ----
            if pieces:
                nc.tensor.matmul(psum_s[0:1, :], ones[:, :], sums[:, :])
                nc.vector.tensor_copy(fin[0:1, 0:nsum], psum_s[0:1, :])
            nc.sync.dma_start(out_fin[:, :], fin[:, :])

    nc.compile()
    return nc


_MODULES = {}


def _module(nrows):
    global PIECES, NSUM
    PIECES = pieces_for(nrows)
    NSUM = max(2 * len(PIECES), 1)
    if nrows not in _MODULES:
        _MODULES[nrows] = build_module(PIECES)
    return _MODULES[nrows]


def nrows_for_mask(mask):
    mask = np.asarray(mask)
    mx = max(int(mask[i * B_LOC : (i + 1) * B_LOC].sum()) * K
             for i in range(N_CORES))
    return ((mx + 63) // 64) * 64


def make_in_maps(count_logits, pred_heatmaps, pred_conf_logits, gt_heatmaps,
                 count, mask, pieces, nrows):
    count_logits = np.asarray(count_logits, np.float32)
    pred_heatmaps = np.asarray(pred_heatmaps, np.float32)
    pred_conf_logits = np.asarray(pred_conf_logits, np.float32)
    gt_heatmaps = np.asarray(gt_heatmaps, np.float32)
    count = np.asarray(count, np.int32)
    mask = np.asarray(mask, np.int32)

    in_maps = []
    for i in range(N_CORES):
        b0, b1 = i * B_LOC, (i + 1) * B_LOC
        mrow = np.repeat(mask[b0:b1].reshape(-1), K)          # [680]
        act = np.nonzero(mrow)[0]
        # gather active rows, pad with zero rows (mask col 0) to nrows
        phl = np.zeros((nrows, COLS), np.float16)
        ghl = np.zeros((nrows, COLS), np.float16)
        mcol = np.zeros((nrows,), np.float16)
        na = len(act)
        phl[:na] = pred_heatmaps[b0:b1].reshape(ROWS, COLS)[act]
        ghl[:na] = gt_heatmaps[b0:b1].reshape(ROWS, COLS)[act]
        mcol[:na] = 1.0

        im = {}
        for name, rr, cc, _, r0, c0, folded in pieces:
            if folded:
                # last 64 rows folded [128, 2048]: partition h*64+r
                pr = phl[r0 : r0 + 64].reshape(64, 2, cc).transpose(1, 0, 2)
                gr = ghl[r0 : r0 + 64].reshape(64, 2, cc).transpose(1, 0, 2)
                pch = pr.reshape(128, cc)
                gch = gr.reshape(128, cc)
                mv = np.concatenate([mcol[r0 : r0 + 64]] * 2)[:, None]
            else:
                pch = phl[r0 : r0 + rr, c0 : c0 + cc]
                gch = ghl[r0 : r0 + rr, c0 : c0 + cc]
                mv = mcol[r0 : r0 + rr][:, None]
            im["p" + name] = np.ascontiguousarray(pch)
            im["g" + name] = np.ascontiguousarray(np.concatenate(
                [gch, mv, 2.0 * mv], axis=1).astype(np.float16))

        smv = np.zeros((B_LOC, 84), np.float32)
        smv[np.arange(B_LOC), 21 + count[b0:b1]] = 1.0       # one-hot
        smv[:, 0:21] = count_logits[b0:b1]
        smv[:, 42:62] = pred_conf_logits[b0:b1]
        smv[:, 62:82] = mask[b0:b1].astype(np.float32)
        im["sm"] = smv
        in_maps.append(im)
    return in_maps


def combine(results, mask):
    mask = np.asarray(mask)
    # stt pieces accumulated 2*mask*Sum(n^2); double those columns
    kappa = np.ones(NSUM)
    for i, (_, _, _, s2_dve, _, _, _) in enumerate(PIECES):
        if s2_dve:
            kappa[2 * i + 1] = 2.0
    hm_sum = 0.0
    ce_sum = 0.0
    fo_sum = 0.0
    ns = 2 * len(PIECES)
    for res in results:
        fin = np.asarray(res["out_fin"], np.float64)
        if ns:
            hm_sum += float(fin[0, :ns] @ kappa[:ns])
        ce_sum += float(fin[:, -3].sum() + fin[:, -2].sum())
        fo_sum += -float(fin[:, -1].sum())
    msum = float(mask.sum())
    hm = hm_sum / (msum * K * H * W + EPS)
    loss_heatmap = hm if msum > 0 else 0.0
    loss_count = ce_sum / B
    loss_conf = fo_sum / (B * P)
    total = (ALPHA_COUNT * loss_count + ALPHA_HEATMAP * loss_heatmap
             + ALPHA_CONF * loss_conf)
    return np.float32(total)


def run(inputs, trace=False, **kwargs):
    """Run on hardware; returns (output_scalar, BassKernelResults)."""
    nrows = nrows_for_mask(inputs["mask"])
    nc = _module(nrows)
    in_maps = make_in_maps(pieces=PIECES, nrows=nrows, **inputs)
    res = bass_utils.run_bass_kernel_spmd(
        nc, in_maps, core_ids=list(range(N_CORES)), trace=trace, **kwargs
    )
    out = combine(res.results, inputs["mask"])
    return out, res


def kernel(count_logits, pred_heatmaps, pred_conf_logits, gt_heatmaps,
           count, mask):
    out, _ = run(dict(
        count_logits=count_logits, pred_heatmaps=pred_heatmaps,
        pred_conf_logits=pred_conf_logits, gt_heatmaps=gt_heatmaps,
        count=count, mask=mask,
    ))
    return out


# revision 42
# speedup vs baseline: 1.0177x; 1.0169x over previous
"""End2EndPoseLoss on 8 Trainium2 NeuronCores.

Data-parallel over batch (B_LOC=2 samples/core). Key structural wins:

1. Mask sparsity, exact: a row's whole heatmap contribution is
   mask * stuff and mask is a kernel input, so each core gathers ONLY
   its mask=1 rows (padded to the max across cores, rounded to 64;
   module compiled per row-count). Seed-0 inputs: 512 of 680 rows.
2. Mask applied on device for free: each g piece carries 2 extra fp16
   columns (mask, 2*mask) used as the ACT Square scale / DVE stt
   scalar, so padding rows vanish and no per-row output is needed.
3. Tiny output: masked sums cross-partition-reduced on the idle PE
   (ones-vector matmul -> PSUM [1, nsum]) and shipped together with
   the small-loss terms as ONE [2, nsum+3] tensor (2 descriptors).
   Avoids multi-descriptor outputs whose completion-semaphore drain
   costs ~25ns/tick at kernel end.
4. Column-halved [128, 2048] pieces: first ACT work starts ~8us
   earlier and the final dependency chain is short.

Per piece: DVE step=(g>0.2) (TS 4x), d=p-g (TT 2x), n=step*d (TT 2x);
ACT Square(mask*d)+accum -> col 2i; col 2i+1 = ACT Square(2mask*n) or,
for the last two pieces, DVE stt (n*2mask)*n+accum (host doubles those
columns) — balances DVE vs ACT busy time.

Measured constraints that shaped this (HW, not the cost model):
  - ts/stt-with-accum run at ~1x (TENSOR_SCALAR_CACHE_REDUCE), never 4x
  - the Pool engine only supports tensor_tensor Add/Mult (no TS/STT),
    and big Pool ops starve DVE via SBUF contention (~2x slowdowns)
  - ACT is exactly 1 elem/clk/lane @1.2GHz regardless of dtype
  - ~7.4us of end-of-kernel semaphore teardown is counted in exec time
    (constant, unavoidable)

Small losses (count CE over [2,21], conf focal over [2,20]) arrive as
one packed [2,84] tensor and run inside the DMA stream.
"""

import sys
import types
import numpy as np

import concourse.bacc as bacc
import concourse.bass as bass  # noqa: F401
import concourse.mybir as mybir
import concourse.tile as tile
from concourse import bass_utils

# Problem constants (hardcoded per contract).
B, P, K, H, W = 16, 20, 17, 64, 64
N_CORES = 8
B_LOC = B // N_CORES            # 2
ROWS = B_LOC * P * K            # 680
COLS = H * W                    # 4096
REM = 40                        # 680 - 5*128

PEAK_THRESH = 0.2
PEAK_WEIGHT = 5.0
ALPHA_COUNT, ALPHA_HEATMAP, ALPHA_CONF = 1.0, 10.0, 1.5
EPS = 1e-6

F32 = mybir.dt.float32
F16 = mybir.dt.float16
ALU = mybir.AluOpType
ACTF = mybir.ActivationFunctionType
AX = mybir.AxisListType

# Only rows with mask=1 are shipped (a row's entire contribution is
# exactly mask * stuff, and mask is a kernel input): each core gathers
# its active rows, padded to the max across cores rounded up to 64.
# pieces: (name, rows, cols, s2_dve = Sum2 via DVE stt instead of ACT
# square); piece i accumulates Sum(d^2) in column 2i, scaled
# Sum(step*d^2) in 2i+1. NROWS % 128 == 64 folds the last 64 rows to
# [128, 2048] (partition h*64+r <-> row r, column half h).
PIECES = []
NSUM = 0


def pieces_for(nrows):
    """Piece list: (name, rows, cols, s2_dve, row0, col0, folded).
    Column-halved [128, 2048] pieces: first data lands sooner and the
    final dependency chain is short. Sum2 runs on DVE stt for the last
    two pieces (~1 chunk-unit), ACT Square otherwise."""
    nf, rem = divmod(nrows, 128)
    assert rem in (0, 64)
    ps = []
    for k in range(nf):
        for h in range(2):
            ps.append(("c%d_%d" % (k, h), 128, COLS // 2, False,
                       k * 128, h * (COLS // 2), False))
    if rem:
        ps.append(("r", 128, COLS // 2, False, nf * 128, 0, True))
    for j in range(max(0, len(ps) - 2), len(ps)):
        nm, rr, cc, _, r0, c0, fo = ps[j]
        ps[j] = (nm, rr, cc, True, r0, c0, fo)
    return ps


def _install_ntff_hook():
    """Provide antenv.axon_hooks if the image lacks it, so that
    run_bass_kernel_spmd(trace=True) doesn't crash and, when possible,
    actually profiles via the axon .so."""
    try:
        from antenv.axon_hooks import get_axon_ntff_profile_hook  # noqa: F401
        return
    except ImportError:
        pass
    try:
        import antenv
    except ImportError:
        return
    import contextlib
    import ctypes

    mod = types.ModuleType("antenv.axon_hooks")
    _h = [None]
    mod.set_axon_ntff_profile_hook = lambda h: _h.__setitem__(0, h)
    mod.get_axon_ntff_profile_hook = lambda: _h[0]
    sys.modules["antenv.axon_hooks"] = mod
    antenv.axon_hooks = mod

    so_path = "/opt/axon/libaxon_pjrt.so"
    try:
        lib = ctypes.CDLL(so_path)
        if not hasattr(lib, "axon_start_nrt_profile"):
            return
        lib.axon_start_nrt_profile.argtypes = [
            ctypes.POINTER(ctypes.c_int64),
            ctypes.c_size_t,
        ]
        lib.axon_start_nrt_profile.restype = ctypes.c_int64
        lib.axon_stop_nrt_profile.argtypes = [ctypes.c_char_p]
        lib.axon_stop_nrt_profile.restype = ctypes.c_int64
    except OSError:
        return

    @contextlib.contextmanager
    def _hook(output_dir, device_ids):
        import jax

        jax.devices()
        if device_ids:
            ids = (ctypes.c_int64 * len(device_ids))(*device_ids)
            rc = lib.axon_start_nrt_profile(ids, len(device_ids))
        else:
            rc = lib.axon_start_nrt_profile(None, 0)
        if rc != 0:
            raise RuntimeError(f"axon_start_nrt_profile rc={rc}")
        try:
            yield
        finally:
            n = lib.axon_stop_nrt_profile(str(output_dir).encode())
            print(f"profile: {n} file(s) written to {output_dir}", file=sys.stderr)

    mod.set_axon_ntff_profile_hook(_hook)


_install_ntff_hook()

_orig_upload = bass_utils.upload_artifacts


def _safe_upload(tmpdir):
    try:
        return _orig_upload(tmpdir)
    except Exception:
        return tmpdir


bass_utils.upload_artifacts = _safe_upload


def build_module(pieces):
    nsum = max(2 * len(pieces), 1)
    nc = bacc.Bacc("TRN2", target_bir_lowering=False, debug=False)

    srcs = {}
    for name, rr, cc, _, _, _, _ in pieces:
        # g pieces carry 2 extra columns: mask and 2*mask for that
        # partition's row (applied on device as ACT scale / stt scalar)
        srcs["g" + name] = nc.dram_tensor("g" + name, [rr, cc + 2], F16, kind="ExternalInput")
        srcs["p" + name] = nc.dram_tensor("p" + name, [rr, cc], F16, kind="ExternalInput")
    sm = nc.dram_tensor("sm", [B_LOC, 84], F32, kind="ExternalInput")

    # single tiny output: row 0 cols 0:nsum = mask-weighted piece sums,
    # cols nsum:nsum+3 (both rows) = pre, ln(se), focal accum
    out_fin = nc.dram_tensor("out_fin", [B_LOC, nsum + 3], F32, kind="ExternalOutput")

    with tile.TileContext(nc) as tc:
        with (
            tc.tile_pool(name="io", bufs=5) as io,
            tc.tile_pool(name="wk", bufs=4) as wk,
            tc.tile_pool(name="acc", bufs=1) as accp,
            tc.tile_pool(name="small", bufs=1) as small,
            tc.tile_pool(name="ps", bufs=1, space=bass.MemorySpace.PSUM) as ps,
        ):
            sums = accp.tile([128, nsum], F32, tag="sums")
            ones = accp.tile([128, 1], F32, tag="ones")
            psum_s = ps.tile([1, nsum], F32, tag="pt")
            nc.gpsimd.memset(sums[:], 0.0)
            nc.gpsimd.memset(ones[:], 1.0)
            # Dummy Exp/Ln on one element: forces both ACT tables (each
            # set also contains Square) to load during the DMA ramp
            # instead of stalling the ACT stream later.
            scr = accp.tile([1, 2], F32, tag="scr")
            nc.scalar.activation(scr[0:1, 0:1], ones[0:1, 0:1], ACTF.Exp)
            nc.scalar.activation(scr[0:1, 1:2], ones[0:1, 0:1], ACTF.Ln)

            # ---- input DMA triggers, g before p so W4 can start early ----
            gts, pts = [], []
            for i, (name, rr, cc, _, _, _, _) in enumerate(pieces):
                g = io.tile([128, COLS + 2], F16, tag="g")
                p = io.tile([128, COLS], F16, tag="p")
                if i == 0:
                    smt = small.tile([B_LOC, 84], F32, tag="sm")
                    nc.sync.dma_start(smt[:], sm[:, :])
                nc.sync.dma_start(g[:rr, :cc + 2], srcs["g" + name][:, :])
                nc.sync.dma_start(p[:rr, :cc], srcs["p" + name][:, :])
                gts.append(g)
                pts.append(p)

            def heavy(i):
                name, rr, cc, s2_dve = pieces[i][:4]
                c1, c2 = 2 * i, 2 * i + 1
                g, p = gts[i], pts[i]
                st = wk.tile([128, COLS], F16, tag="st")
                d = wk.tile([128, COLS], F16, tag="d")
                n = wk.tile([128, COLS], F16, tag="n")
                # step = (g > thresh)          (TS, 4x)
                nc.vector.tensor_scalar(
                    st[:rr, :cc], g[:rr, :cc], float(PEAK_THRESH), None,
                    op0=ALU.is_gt,
                )
                # d = p - g                    (TT, 2x)
                nc.vector.tensor_sub(d[:rr, :cc], p[:rr, :cc], g[:rr, :cc])
                # n = step * d                 (TT, 2x)
                nc.vector.tensor_mul(n[:rr, :cc], st[:rr, :cc], d[:rr, :cc])
                mk = wk.tile([128, 2], F32, tag="mk")
                nc.vector.tensor_copy(mk[:rr, :], g[:rr, cc : cc + 2])
                mcol = mk[:rr, 0:1]               # mask {0,1}
                m2col = mk[:rr, 1:2]              # 2*mask {0,2}
                # mask*Sum(d^2) on ACT: Square(mask*d), accumulator -> col 2i
                nc.scalar.activation(
                    d[:rr, :cc], d[:rr, :cc], ACTF.Square, scale=mcol,
                    accum_out=sums[:rr, c1 : c1 + 1],
                )
                # col 2i+1: ACT Square(2*mask*n) = 4*mask*Sum(n^2), or DVE
                # stt (n*2mask)*n = 2*mask*Sum(n^2) (host doubles those cols)
                if s2_dve:
                    nc.vector.scalar_tensor_tensor(
                        out=g[:rr, :cc], in0=n[:rr, :cc], scalar=m2col,
                        in1=n[:rr, :cc], op0=ALU.mult, op1=ALU.mult,
                        accum_out=sums[:rr, c2 : c2 + 1],
                    )
                else:
                    nc.scalar.activation(
                        n[:rr, :cc], n[:rr, :cc], ACTF.Square, scale=m2col,
                        accum_out=sums[:rr, c2 : c2 + 1],
                    )

            # ---- small losses (inside the DMA stream) ----
            cl_t = smt[:, 0:21]
            oh_t = smt[:, 21:42]
            lt_ = smt[:, 42:62]
            tt_ = smt[:, 62:82]

            # count cross-entropy pieces
            mx = small.tile([B_LOC, 1], F32, tag="mx")
            nc.vector.tensor_reduce(mx[:], cl_t, axis=AX.X, op=ALU.max)
            nmx = small.tile([B_LOC, 1], F32, tag="nmx")
            nc.vector.tensor_scalar_mul(nmx[:], mx[:], -1.0)
            et = small.tile([B_LOC, 21], F32, tag="et")
            se = small.tile([B_LOC, 1], F32, tag="se")
            nc.scalar.activation(
                et[:], cl_t, ACTF.Exp, bias=nmx[:], scale=1.0, accum_out=se[:]
            )
            junk21 = small.tile([B_LOC, 21], F32, tag="junk21")
            tg = small.tile([B_LOC, 1], F32, tag="tg")
            nc.vector.scalar_tensor_tensor(
                out=junk21[:], in0=cl_t, scalar=1.0, in1=oh_t,
                op0=ALU.mult, op1=ALU.mult, accum_out=tg[:],
            )
            fin = small.tile([B_LOC, nsum + 3], F32, tag="fin")
            nc.gpsimd.memset(fin[:], 0.0)
            nc.vector.tensor_sub(fin[:, nsum : nsum + 1], mx[:], tg[:])

            # focal: p_t = 1 - |t - sigma(l)| with sigma from exp(-|l|)
            ab = small.tile([B_LOC, P], F32, tag="ab")
            nc.vector.scalar_tensor_tensor(
                out=ab[:], in0=lt_, scalar=-1.0, in1=lt_,
                op0=ALU.mult, op1=ALU.max,
            )
            z = small.tile([B_LOC, P], F32, tag="z")
            nc.scalar.activation(z[:], ab[:], ACTF.Exp, scale=-1.0)
            zz = small.tile([B_LOC, P], F32, tag="zz")
            nc.vector.tensor_scalar(zz[:], z[:], 1.0, None, op0=ALU.add)
            r = small.tile([B_LOC, P], F32, tag="r")
            nc.vector.reciprocal(r[:], zz[:])          # sigma(|l|)
            sgn = small.tile([B_LOC, P], F32, tag="sgn")
            nc.vector.tensor_scalar(sgn[:], lt_, 0.0, None, op0=ALU.is_ge)
            t1 = small.tile([B_LOC, P], F32, tag="t1")
            nc.vector.tensor_scalar(t1[:], r[:], 2.0, -1.0, op0=ALU.mult, op1=ALU.add)
            t2 = small.tile([B_LOC, P], F32, tag="t2")
            nc.vector.tensor_scalar(t2[:], r[:], -1.0, 1.0, op0=ALU.mult, op1=ALU.add)
            sl0 = small.tile([B_LOC, P], F32, tag="sl0")
            nc.vector.scalar_tensor_tensor(
                out=sl0[:], in0=sgn[:], scalar=1.0, in1=t1[:],
                op0=ALU.mult, op1=ALU.mult,
            )
            sig = small.tile([B_LOC, P], F32, tag="sig")
            nc.gpsimd.tensor_add(sig[:], sl0[:], t2[:])
            u = small.tile([B_LOC, P], F32, tag="u")
            nc.gpsimd.tensor_sub(u[:], tt_, sig[:])
            au = small.tile([B_LOC, P], F32, tag="au")
            nc.vector.scalar_tensor_tensor(
                out=au[:], in0=u[:], scalar=-1.0, in1=u[:],
                op0=ALU.mult, op1=ALU.max,
            )
            pt = small.tile([B_LOC, P], F32, tag="pt")
            nc.vector.tensor_scalar(pt[:], au[:], -1.0, 1.0, op0=ALU.mult, op1=ALU.add)
            au2 = small.tile([B_LOC, P], F32, tag="au2")
            nc.gpsimd.tensor_mul(au2[:], au[:], au[:])

            lnz = small.tile([B_LOC, 1], F32, tag="lnz")
            nc.scalar.activation(lnz[:], se[:], ACTF.Ln)
            nc.vector.tensor_copy(fin[:, nsum + 1 : nsum + 2], lnz[:])
            lnpt = small.tile([B_LOC, P], F32, tag="lnpt")
            nc.scalar.activation(lnpt[:], pt[:], ACTF.Ln)
            junk20 = small.tile([B_LOC, P], F32, tag="junk20")
            fr = small.tile([B_LOC, 1], F32, tag="fr")
            # accum = sum(au^2 * ln(p_t)) = -focal_sum   (host negates)
            nc.vector.scalar_tensor_tensor(
                out=junk20[:], in0=au2[:], scalar=1.0, in1=lnpt[:],
                op0=ALU.mult, op1=ALU.mult, accum_out=fr[:],
            )
            nc.vector.tensor_copy(fin[:, nsum + 2 : nsum + 3], fr[:])

            # ---- heavy pieces (smalls above fill the DMA ramp) ----
            for i in range(len(pieces)):
                heavy(i)

            # ---- cross-partition reduce of masked sums on PE, 1 tiny DMA ----
            if pieces:
                nc.tensor.matmul(psum_s[0:1, :], ones[:, :], sums[:, :])
                nc.vector.tensor_copy(fin[0:1, 0:nsum], psum_s[0:1, :])
            nc.sync.dma_start(out_fin[:, :], fin[:, :])

    nc.compile()
    return nc


_MODULES = {}


def _module(nrows):
    global PIECES, NSUM
    PIECES = pieces_for(nrows)
    NSUM = max(2 * len(PIECES), 1)
    if nrows not in _MODULES:
        _MODULES[nrows] = build_module(PIECES)
    return _MODULES[nrows]


def nrows_for_mask(mask):
    mask = np.asarray(mask)
    mx = max(int(mask[i * B_LOC : (i + 1) * B_LOC].sum()) * K
             for i in range(N_CORES))
    return ((mx + 63) // 64) * 64


def make_in_maps(count_logits, pred_heatmaps, pred_conf_logits, gt_heatmaps,
                 count, mask, pieces, nrows):
    count_logits = np.asarray(count_logits, np.float32)
    pred_heatmaps = np.asarray(pred_heatmaps, np.float32)
    pred_conf_logits = np.asarray(pred_conf_logits, np.float32)
    gt_heatmaps = np.asarray(gt_heatmaps, np.float32)
    count = np.asarray(count, np.int32)
    mask = np.asarray(mask, np.int32)

    in_maps = []
    for i in range(N_CORES):
        b0, b1 = i * B_LOC, (i + 1) * B_LOC
        mrow = np.repeat(mask[b0:b1].reshape(-1), K)          # [680]
        act = np.nonzero(mrow)[0]
        # gather active rows, pad with zero rows (mask col 0) to nrows
        phl = np.zeros((nrows, COLS), np.float16)
        ghl = np.zeros((nrows, COLS), np.float16)
        mcol = np.zeros((nrows,), np.float16)
        na = len(act)
        phl[:na] = pred_heatmaps[b0:b1].reshape(ROWS, COLS)[act]
        ghl[:na] = gt_heatmaps[b0:b1].reshape(ROWS, COLS)[act]
        mcol[:na] = 1.0

        im = {}
        for name, rr, cc, _, r0, c0, folded in pieces:
            if folded:
                # last 64 rows folded [128, 2048]: partition h*64+r
                pr = phl[r0 : r0 + 64].reshape(64, 2, cc).transpose(1, 0, 2)
                gr = ghl[r0 : r0 + 64].reshape(64, 2, cc).transpose(1, 0, 2)
                pch = pr.reshape(128, cc)
                gch = gr.reshape(128, cc)
                mv = np.concatenate([mcol[r0 : r0 + 64]] * 2)[:, None]
            else:
                pch = phl[r0 : r0 + rr, c0 : c0 + cc]
                gch = ghl[r0 : r0 + rr, c0 : c0 + cc]
                mv = mcol[r0 : r0 + rr][:, None]
            im["p" + name] = np.ascontiguousarray(pch)
            im["g" + name] = np.ascontiguousarray(np.concatenate(
                [gch, mv, 2.0 * mv], axis=1).astype(np.float16))

        smv = np.zeros((B_LOC, 84), np.float32)
        smv[np.arange(B_LOC), 21 + count[b0:b1]] = 1.0       # one-hot
        smv[:, 0:21] = count_logits[b0:b1]
        smv[:, 42:62] = pred_conf_logits[b0:b1]
        smv[:, 62:82] = mask[b0:b1].astype(np.float32)
        im["sm"] = smv
        in_maps.append(im)
    return in_maps


def combine(results, mask):
    mask = np.asarray(mask)
    # stt pieces accumulated 2*mask*Sum(n^2); double those columns
    kappa = np.ones(NSUM)
    for i, (_, _, _, s2_dve, _, _, _) in enumerate(PIECES):
        if s2_dve:
            kappa[2 * i + 1] = 2.0
    hm_sum = 0.0
    ce_sum = 0.0
    fo_sum = 0.0
    ns = 2 * len(PIECES)
    for res in results:
        fin = np.asarray(res["out_fin"], np.float64)
        if ns:
            hm_sum += float(fin[0, :ns] @ kappa[:ns])
        ce_sum += float(fin[:, -3].sum() + fin[:, -2].sum())
        fo_sum += -float(fin[:, -1].sum())
    msum = float(mask.sum())
    hm = hm_sum / (msum * K * H * W + EPS)
    loss_heatmap = hm if msum > 0 else 0.0
    loss_count = ce_sum / B
    loss_conf = fo_sum / (B * P)
    total = (ALPHA_COUNT * loss_count + ALPHA_HEATMAP * loss_heatmap
             + ALPHA_CONF * loss_conf)
    return np.float32(total)


def run(inputs, trace=False, **kwargs):
    """Run on hardware; returns (output_scalar, BassKernelResults)."""
    nrows = nrows_for_mask(inputs["mask"])
    nc = _module(nrows)
    in_maps = make_in_maps(pieces=PIECES, nrows=nrows, **inputs)
    res = bass_utils.run_bass_kernel_spmd(
        nc, in_maps, core_ids=list(range(N_CORES)), trace=trace, **kwargs
    )
    out = combine(res.results, inputs["mask"])
    return out, res


def kernel(count_logits, pred_heatmaps, pred_conf_logits, gt_heatmaps,
           count, mask):
    out, _ = run(dict(
        count_logits=count_logits, pred_heatmaps=pred_heatmaps,
        pred_conf_logits=pred_conf_logits, gt_heatmaps=gt_heatmaps,
        count=count, mask=mask,
    ))
    return out
